# revision 15
# baseline (speedup 1.0000x reference)
"""Ball-query KNN (first K=10 neighbors within radius, ascending index) on 8 TRN2 cores.

Algorithm per core (queries sharded across cores, point cloud replicated):
  - One K=5 matmul computes the validity score s'[q,j] = r^2 - |q|^2 + 2 q.x_j - |x_j|^2
    (s' >= 0  <=>  candidate j is within radius of query q).
  - ACT Sign turns s' into {-1,0,+1}; a fused scalar_tensor_tensor builds
    key[q,j] = (sign >= 0) * (2^24 - j)  -- valid candidates get a key that is
    strictly decreasing in j, invalid ones get 0.
  - DVE max8 -> match_replace -> max8 extracts the 16 largest keys per row
    = the 16 smallest valid candidate indices. A query is complete iff >= 10
    of its slots are valid.
  - Host decodes j = 2^24 - key and merges. Queries still incomplete after the
    first window (candidates [0,512)) are re-run on window [512,2048), then
    [2048,8192). Window widths chosen so ~98% of queries finish in call A.

Self-contained: shapes/constants hardcoded for x[1,8192,3], p_grid[1,64,32,16,3].
"""

import os
import numpy as np

NCORES = 8
NX = 8192
NQ_TOT = 32768
K = 10
RADIUS = 0.25
R2 = np.float32(RADIUS * RADIUS)
HUGE = 16777216.0  # 2^24; keys are HUGE - j, exact in fp32

# (ntiles_per_core, window_width, window_base); call A covers [0,512) for all
# queries, B/C mop up stragglers.
CALLS = [
    (32, 512, 0),
    (1, 1536, 512),
    (1, 6144, 2048),
]

_prog_cache = {}
LAST_RUN_INFO = []  # [(label, BassKernelResults), ...] for test harness inspection


def _build(ntiles, W, base):
    import concourse.bass as bass
    import concourse.mybir as mybir
    import concourse.tile as tile

    F32 = mybir.dt.float32
    BF16 = mybir.dt.bfloat16
    AF = mybir.ActivationFunctionType
    ALU = mybir.AluOpType

    nc = bass.Bass()
    NQ = ntiles * 128
    nch = W // 512
    # Single packed const input: qa in cols [0,NQ), xa in cols [NQ,NQ+W).
    # One DMA -> one HW queue semaphore (walrus allows at most 2 sync waits
    # per instruction, so fan-in must stay small).
    qx = nc.declare_dram_parameter("qx", [5, NQ + W], F32, isOutput=False)
    top = nc.declare_dram_parameter("top", [NQ, 16], F32, isOutput=True)

    with tile.TileContext(nc) as tc:
        with tc.tile_pool(name="const", bufs=1) as cpool, \
             tc.tile_pool(name="key", bufs=3) as kpool, \
             tc.tile_pool(name="ps", bufs=4, space="PSUM") as ppool:
            qx_sb = cpool.tile([5, NQ + W], F32)
            nc.sync.dma_start(out=qx_sb[:], in_=qx[:])
            qa_sb = qx_sb[:, 0:NQ]
            xa_sb = qx_sb[:, NQ:NQ + W]
            # Key constants 2^24 - (base + j), generated on-device. DVE iota in
            # f32 is exact (all values < 2^24), and producing it on DVE keeps
            # every consumer's dependency same-engine (merged into one wait).
            kc_sb = cpool.tile([128, W], F32)
            nc.gpsimd.iota(kc_sb[:], pattern=[[-1, W]], base=int(HUGE) - base,
                           channel_multiplier=0,
                           allow_small_or_imprecise_dtypes=True)
            # One throwaway DVE read of kc: DVE's vector clock observes the
            # Pool semaphore here, so none of the per-tile DVE ops need a
            # second sync wait for it (walrus limit: 1 wait on these instrs).
            kc_probe = cpool.tile([128, 2], F32)
            nc.vector.tensor_copy(kc_probe[:], kc_sb[:, 0:2])
            # All per-tile results land here; one DMA at the end (per-tile DMAs
            # would add WAR waits on the small t8 tiles).
            t8all = cpool.tile([128, ntiles * 16], F32)

            for i in range(ntiles):
                key = kpool.tile([128, W], F32, tag="key")
                key2 = kpool.tile([128, W], F32, tag="key2")
                for c in range(nch):
                    sl = slice(c * 512, (c + 1) * 512)
                    ps = ppool.tile([128, 512], F32, tag="ps")
                    nc.tensor.matmul(
                        ps[:],
                        qa_sb[:, i * 128:(i + 1) * 128],
                        xa_sb[:, sl],
                        start=True,
                        stop=True,
                    )
                    # key = (s' >= 0) * (2^24 - j); PE -> DVE keeps the
                    # cross-engine fan-in within walrus's 2-sync-wait limit.
                    nc.vector.scalar_tensor_tensor(
                        key[:, sl], ps[:], 0.0, kc_sb[:, sl],
                        op0=ALU.is_ge, op1=ALU.mult,
                    )
                t8a = t8all[:, i * 16:i * 16 + 8]
                t8b = t8all[:, i * 16 + 8:i * 16 + 16]
                nc.vector.max(out=t8a, in_=key[:])
                nc.vector.match_replace(
                    out=key2[:], in_to_replace=t8a, in_values=key[:],
                    imm_value=-1.0,
                )
                nc.vector.max(out=t8b, in_=key2[:])
                nc.sync.dma_start(
                    out=top[i * 128:(i + 1) * 128, :],
                    in_=t8all[:, i * 16:(i + 1) * 16],
                )
    _split_wide_waits(nc)
    return nc


def _split_wide_waits(nc, limit=1):
    """walrus's codegen allows very few sync waits per instruction (one, for
    some instruction structs). Hoist excess waits onto NoOps on the same
    engine placed immediately before the offending instruction — execution
    order per engine makes this semantically identical."""
    import concourse.mybir as mybir

    for fn in nc.m.functions:
        for bb in fn.blocks:
            il = bb.instructions
            i = 0
            while i < len(il):
                inst = il[i]
                si = inst.sync_info
                if si is not None and len(si.on_wait) > limit:
                    waits = list(si.on_wait)
                    keep, extra = waits[-limit:], waits[:-limit]
                    nops = []
                    for w0 in range(0, len(extra), limit):
                        nop = mybir.InstNoOp(
                            name=f"{inst.name}-wsplit{w0}", ins=[], outs=[])
                        nop.engine = inst.engine
                        nop.sync_info = mybir.SyncInfo(
                            on_wait=extra[w0:w0 + limit], on_update=[])
                        nc.register_instruction(nop, overwrite=True)
                        nops.append(nop)
                    inst.sync_info = mybir.SyncInfo(
                        on_wait=keep, on_update=si.on_update)
                    il[i:i] = nops
                    i += len(nops)
                i += 1


def _get_prog(ntiles, W, base):
    key = (ntiles, W, base)
    if key not in _prog_cache:
        _prog_cache[key] = _build(ntiles, W, base)
    return _prog_cache[key]


def _run_call(ntiles, W, base, qsel, qa_all, xa_full, sim=False):
    """qsel: [NCORES, ntiles*128] global query indices. Returns tops [NCORES*ntiles*128, 16]."""
    xa_win = np.ascontiguousarray(xa_full[:, base:base + W])
    if sim:
        kcrow = (HUGE - (base + np.arange(W))).astype(np.float32)
        outs = []
        for c in range(NCORES):
            qa_c = qa_all[:, qsel[c]]                       # [5, NQ]
            s = qa_c.T.astype(np.float32) @ xa_win.astype(np.float32)  # [NQ, W]
            keyv = (np.sign(s) >= 0).astype(np.float32) * kcrow[None, :]
            top16 = -np.sort(-keyv, axis=1)[:, :16]
            outs.append(top16.astype(np.float32))
        return np.concatenate(outs, 0)

    from concourse.bass_utils import run_bass_kernel_spmd

    nc = _get_prog(ntiles, W, base)
    in_maps = [
        {"qx": np.ascontiguousarray(
            np.concatenate([qa_all[:, qsel[c]], xa_win], axis=1))}
        for c in range(NCORES)
    ]
    trace = bool(int(os.environ.get("BQ_TRACE", "0")))
    res = run_bass_kernel_spmd(nc, in_maps, list(range(NCORES)), trace=trace)
    LAST_RUN_INFO.append(((ntiles, W, base), res))
    return np.concatenate([res.results[c]["top"] for c in range(NCORES)], 0)


def _merge(hits, cnt, qidx, tops):
    """Decode a call's top-16 keys and append the hits for queries qidx."""
    gj = np.rint(HUGE - tops.astype(np.float64)).astype(np.int64)  # [n, 16]
    validm = (gj >= 0) & (gj < NX)  # valid slots form a prefix of each row
    nvalid = validm.sum(1)
    take = np.clip(np.minimum(K - cnt[qidx], nvalid), 0, None)
    colidx = np.arange(16)[None, :]
    m = colidx < take[:, None]
    rows_q = np.repeat(qidx, take)
    slots = (cnt[qidx][:, None] + colidx)[m]
    hits[rows_q, slots] = gj[m]
    cnt[qidx] += take


def kernel(x, p_grid):
    sim = bool(int(os.environ.get("BQ_SIM", "0")))
    xc = np.asarray(x, np.float32).reshape(NX, 3)
    xq = np.asarray(p_grid, np.float32).reshape(NQ_TOT, 3)

    q2 = (xq * xq).sum(1, dtype=np.float32)
    x2 = (xc * xc).sum(1, dtype=np.float32)
    qa = np.ascontiguousarray(
        np.stack([xq[:, 0], xq[:, 1], xq[:, 2], R2 - q2, np.ones_like(q2)], 0)
    ).astype(np.float32)                                    # [5, NQ_TOT]
    xa = np.ascontiguousarray(
        np.stack([2 * xc[:, 0], 2 * xc[:, 1], 2 * xc[:, 2],
                  np.ones_like(x2), -x2], 0)
    ).astype(np.float32)                                    # [5, NX]

    LAST_RUN_INFO.clear()
    hits = np.zeros((NQ_TOT, K), np.int64)
    cnt = np.zeros(NQ_TOT, np.int64)

    ntiles, W, base = CALLS[0]
    qsel = np.arange(NQ_TOT).reshape(NCORES, -1)
    tops = _run_call(ntiles, W, base, qsel, qa, xa, sim=sim)
    _merge(hits, cnt, qsel.ravel(), tops)

    for ntiles, W, base in CALLS[1:]:
        strag = np.where(cnt < K)[0]
        if len(strag) == 0:
            break
        cap = NCORES * ntiles * 128
        for off in range(0, len(strag), cap):
            batch = strag[off:off + cap]
            pad = np.full(cap, batch[0], np.int64)
            pad[:len(batch)] = batch
            qsel_b = pad.reshape(NCORES, -1)
            tops = _run_call(ntiles, W, base, qsel_b, qa, xa, sim=sim)
            _merge(hits, cnt, batch, tops[:len(batch)])

    mapping = hits  # unfilled slots stay 0, matching the reference
    found = np.arange(K)[None, :] < cnt[:, None]
    outputs = xc[mapping] * found[..., None].astype(np.float32)
    return (
        mapping.reshape(1, NQ_TOT, K).astype(np.int32),
        outputs.reshape(1, NQ_TOT, K, 3).astype(np.float32),
    )
